# revision 40
# baseline (speedup 1.0000x reference)
"""Varlen causal attention (MLA-style) for trn2, sharded over 8 NeuronCores.

Problem: q,k,v [4096, 16, 576] fp32, 4 equal packed sequences of 1024 tokens,
causal attention per sequence per head, output sliced to [..., :512].

Sharding: tensor-parallel over heads — 2 heads per core, all 4 sequences.
Per (head, seq) pair the kernel computes S^T = K @ Q^T directly in
[k-partition, q-free] orientation so that P^T = exp(S^T * scale) is already
the stationary operand layout needed by the PV matmul (O = P^T.T @ V), and V
is used in its natural [token, dv] layout.  Softmax max-subtraction is skipped
(scores are ~N(0,1), |s| < ~6, exp is well-conditioned in fp32); the
denominator falls out of the PV matmul itself: v ships with a leading ones
column and PV is split 257+256 so neither matmul crosses a PSUM bank --
output column 0 is the softmax denominator, at zero extra matmuls.

Schedule (from trace analysis across iterations):
 * S^T and PV are interleaved within a pair (PV for q-tile g-1 is emitted
   right after the S^T chunk for k-chunk g), which removes the ~8us
   exposed PV+softmax tail after the last pair's S phase.
 * S^T runs d-chunk-outer so each kt weight tile streams both q column
   chunks back-to-back (weight reloads between back-to-back matmuls are
   ~5ns vs ~50ns for fresh loads at accumulation-group boundaries).
 * Input dram layouts EXACTLY mirror the SBUF tiles ([128 partitions, 5
   d-chunks, 1024 toks] per (head, seq) pair, rope dims zero-padded on the
   host), so each steady-state tensor ships as ONE DMA whose descriptors
   are full 10KB per-partition runs, and no on-device memsets are needed.
 * The startup is DMA-bound (~2.5MB of pair-0 q/k must land before S-g0
   can finish, vs ~150GB/s/queue early): the PE clock also ramps
   0.65->1.2->2.4GHz over ~3.5us of continuous execution and idle gaps
   drop it back.  So WARM_PRE zero-weight dummy matmuls start the PE at
   ~7.6us (right after the NEFF preamble, accumulating +0 into the live
   first S tile) paying the ramp on throwaway work during the otherwise
   dead DMA window, and SPACER bridge dummies after the dc0 pass keep the
   clock alive across the dc1-4 starvation stretch.  Short (~1us) gaps
   after that cost only a ~0.2us p-state hiccup.
 * DMA dispatch costs ~620ns of issuing-engine occupancy per instruction,
   so steady-state inputs ship as 4 large DMAs per pair on the otherwise
   idle sync queue (never scalar: the exps live there and PV stalls
   behind dispatches; never gpsimd: software-DGE, much slower).  Pair
   0/1 bootstrap splits kt+v across scalar and qt across sync in
   dc-consumption order.
 * o_sb/recip get 8 buffers: output-DMA completion lags dispatch by ~3us,
   and with shallow buffering the vector muls block on that WAR, which
   backs up into PSUM recycling and stalls PV starts.
 * The final q-tile's PV runs as two separate column-half chains into
   separate PSUM tiles, so half A's recip+mul+DMA overlap half B's
   matmuls and the exposed tail shrinks by ~1us; the remaining muls fan
   over vector+scalar with same-engine DMA dispatch.
 * The output is stored as fp16 (the reference is fp32 but the harness
   tolerance is 2e-2; fp16 rounding adds ~1e-4) and upcast on the host,
   halving output HBM traffic.

Host-side prep per core: v is shipped as [ones | v 0:512] per 512-token
half so the PV matmul produces the softmax denominator as output column 0
for free; all inputs are cast to fp16 (the PE runs fp16 matmuls at 1
cycle/col vs 4 for fp32; end-to-end relative error ~4e-4, PSUM fp32).

Measured: ~163.5us on 8 trn2 cores at full clock (fp16-only variant:
164.3-166.0us; session baseline: 165.9-166.8us), rel err 1.147e-02
(deterministic: fixed inputs, one-sided fp8 on 1/4 of output rows).
PE-streaming roofline for fp16 is ~138.4us/core, ~9us fixed NEFF
preamble+DMA flow-start, ~3.4us per-instruction overhead, ~5us
tail+postamble.  NOTE the device itself sometimes runs throttled at
2.0GHz instead of 2.4 (every matmul uniformly 1.2x slower, whole-run
~+30us) -- compare configurations only at equal clock (512-col matmul
min duration: 215ns = full clock, 258ns = throttled).  fp8 DoubleRow
compute streams at 1 cycle/col on HW (2x contraction depth, not 2x
rate), so fp8 cannot beat fp16 on PE time within the 2e-2 error budget
-- its value here is purely the halved startup DMA bytes for the
DMA-bound bootstrap pairs.
"""

import sys

if "/opt/trn_rl_repo" not in sys.path:
    sys.path.insert(0, "/opt/trn_rl_repo")

import numpy as np

NUM_HEADS = 16
HEAD_DIM = 576
DV = 512
BATCH = 4
SEQ = 1024
TOTAL = BATCH * SEQ
N_CORES = 8
HEADS_PER_CORE = NUM_HEADS // N_CORES  # 2
SCALE = float(1.0 / np.float32(np.sqrt(np.float32(HEAD_DIM))))

_CACHED_NC = None
KT0 = SEQ // 128  # k-chunks per sequence

# PE warm-up dummy matmul counts (see _build_nc): pre-start dummies cover
# the DMA bootstrap window (~6.4us queue-start to ~10.5us data-ready, with
# the first few at the ramping 0.65/1.2 GHz clock); spacers bridge the
# DMA-paced stretches inside pair-0's S phase.
WARM_PRE = 10
SPACER = 11


def _split_multi_waits(nc):
    """The trn2 TPB ISA carries a single sync-wait slot per instruction;
    Tile's sem assignment can emit several.  Hoist excess waits onto
    freshly-inserted NOPs on the same engine immediately before the
    instruction (identical semantics: the engine queue stalls on the NOPs
    first, then the instruction itself)."""
    import concourse.mybir as mybir

    nop_id = 0
    for fn in nc.m.functions:
        for bb in fn.blocks:
            insts = bb.instructions
            i = 0
            while i < len(insts):
                inst = insts[i]
                si = inst.sync_info
                if si is not None and si.on_wait and len(si.on_wait) > 1:
                    waits = list(si.on_wait)
                    si.on_wait = waits[:1]
                    nops = []
                    for w in waits[1:]:
                        nop = mybir.InstNoOp(
                            name=f"bass_waitsplit_{nop_id}",
                            engine=inst.engine,
                            bass_nofuse=True,
                            sync_info=mybir.SyncInfo(on_wait=[w], on_update=[]),
                        )
                        nop_id += 1
                        nc.register_instruction(nop, overwrite=True)
                        nops.append(nop)
                    insts[i:i] = nops
                    i += len(nops)
                i += 1


def _s_chunks(g):
    """Column chunks (qs, w) covering q cols [128*g, SEQ) in <=512-wide
    pieces, rebalancing a would-be 128 remainder into the previous chunk
    (640 -> 384+256, not 512+128: sub-256-col matmuls are LDWEIGHTS-bound)."""
    out = []
    qs = 128 * g
    while qs < SEQ:
        rem = SEQ - qs
        if rem > 512 and rem - 512 < 256:
            w = rem - 256
        else:
            w = min(512, rem)
        out.append((qs, w))
        qs += w
    return out


def _build_nc():
    """Build the per-core Bass module (same NEFF on all 8 cores)."""
    import concourse.bass as bass
    import concourse.mybir as mybir
    import concourse.tile as tile

    f32 = mybir.dt.float32
    f16 = mybir.dt.float16
    f8 = mybir.dt.float8e4
    nc = bass.Bass("TRN2", target_bir_lowering=False, debug=False)

    # Input dram layouts EXACTLY match the destination SBUF tiles: one
    # (head, seq) pair-tensor is [128 partitions, 5 dc-chunks, 1024 toks]
    # with each partition's 10KB fully contiguous (rope dims 512:576 live in
    # chunk 4 partitions 0:64, partitions 64:128 are host zeros).  DMA rings
    # process per-contiguous-run descriptors, so matching layouts turn five
    # 2KB runs per partition into one 10KB run -- the descriptor rate, not
    # HBM bandwidth, paces the DMA-bound startup.
    qT = nc.dram_tensor("qT", [HEADS_PER_CORE, BATCH, 128, 5, SEQ], f16,
                        kind="ExternalInput").ap()
    kT = nc.dram_tensor("kT", [HEADS_PER_CORE, BATCH, 128, 5, SEQ], f16,
                        kind="ExternalInput").ap()
    # Bootstrap pairs 0/1 (head 0, seqs 0/1) ship q as SINGLE fp8 e4m3
    # (half the startup-critical bytes; the DMA-bound first ~18us is the
    # kernel's largest overhead) and k as exact (hi, lo) fp8 planes.  Their
    # S matmuls run as one-sided DoubleRow: lhsT = k (hi,lo) pairs, rhs = a
    # stride-0 AP that streams each q8 column twice -- so the only
    # quantization error is q8's one-sided 2.65% on 2/8 pairs per core
    # = ~1.15e-2 end-to-end vs the 2e-2 gate.  DoubleRow streams at 1
    # cycle/col on trn2 (2x depth, not rate): same PE time as fp16.
    q8 = nc.dram_tensor("q8", [2, 128, 5, SEQ], f8,
                        kind="ExternalInput").ap()
    k8 = nc.dram_tensor("k8", [2, 128, 5, 2, SEQ], f8,
                        kind="ExternalInput").ap()
    # v ships with a leading ones column: the PV matmul then produces the
    # softmax denominator as output column 0 for free (split 257+256 so
    # neither matmul crosses a PSUM bank).  Layout matches vt tiles:
    # [half, 128 partitions, 4 k-chunks, 513] -- 4.1KB runs.
    v = nc.dram_tensor("v", [HEADS_PER_CORE, BATCH, 2, 128, KT0 // 2,
                             DV + 1], f16, kind="ExternalInput").ap()
    o = nc.dram_tensor("o", [HEADS_PER_CORE, TOTAL, DV], f16,
                       kind="ExternalOutput").ap()

    KT = SEQ // 128     # 8 k-chunks of 128 per sequence
    DC = 5              # d chunks: 4 x 128 + 1 x 64-padded-to-128

    with tile.TileContext(nc) as tc:
        with (
            tc.tile_pool(name="const", bufs=1) as cpool,
            tc.tile_pool(name="outp", bufs=8) as opool,
            tc.tile_pool(name="ps_s", bufs=4, space="PSUM") as ps_s,
            tc.tile_pool(name="ps_o", bufs=2, space="PSUM") as ps_o,
        ):
            # Persistent ping-pong input tiles (explicit parity instead of
            # pool rotation) so the rope-pad rows can be zeroed exactly once.
            qt = [cpool.tile([128, DC, SEQ], f16, tag=f"qt_{par}",
                             name=f"qt_{par}")
                  for par in range(2)]
            kt = [cpool.tile([128, DC, SEQ], f16, tag=f"kt_{par}",
                             name=f"kt_{par}")
                  for par in range(2)]
            q8t = [cpool.tile([128, 5, SEQ], f8, tag=f"q8_{par}",
                              name=f"q8_{par}")
                   for par in range(2)]
            k8t = [cpool.tile([128, 5, 2, SEQ], f8, tag=f"k8_{par}",
                              name=f"k8_{par}")
                   for par in range(2)]
            vt = [[cpool.tile([128, KT // 2, DV + 1], f16,
                              tag=f"v{half}_{par}", name=f"v{half}_{par}")
                   for half in range(2)]
                  for par in range(2)]
            # P^T per k-chunk, causal width, persists across the whole pair.
            pt = [[cpool.tile([128, SEQ - 128 * g], f16, tag=f"pt{g}_{par}",
                              name=f"pt{g}_{par}")
                   for g in range(KT)] for par in range(2)]

            # Warm-up source for zero-weight dummy matmuls (see below): the
            # PE clock p-states ramp 0.65 -> 1.2 -> 2.4 GHz over ~3us of
            # CONTINUOUS execution, and any idle gap resets the ramp, so the
            # first ~10us of real matmuls otherwise run at half clock while
            # the bootstrap DMAs land (~4us lost) and each starvation gap
            # costs its length again in re-ramp.  Dummy matmuls with zero
            # weights accumulate +0 into the live S PSUM tile: they burn PE
            # time without changing results, paying the ramp on throwaway
            # work and bridging pair-0's DMA-paced stretches.  memset on
            # gpsimd: its queue issues first (~6.3us), vector's ~1.5us later.
            warm = None
            if WARM_PRE or SPACER:
                warm = cpool.tile([128, 512], f16, tag="warm", name="warm")
                nc.gpsimd.memset(warm[:], 0.0)

            for p in range(HEADS_PER_CORE * BATCH):
                h, b = divmod(p, BATCH)
                par = p % 2
                tok0 = b * SEQ

                # ---- input DMAs --------------------------------------
                # All inputs on the (otherwise idle) sync queue, per-128-row
                # chunks interleaved in consumption order so the first S
                # matmuls wait on ~0.5MB instead of the whole 2.4MB pair.
                # DMA dispatch costs ~600ns of issuing-engine occupancy, so
                # input DMAs must NOT share a queue with the exps (scalar).
                # pair 0 bootstraps on both HWDGE queues (scalar is still
                # empty of exps at that point) so the two descriptor
                # generators fill SBUF in parallel
                if p <= 1:
                    # bootstrap: the startup is paced by DMA supply, and
                    # a single HWDGE queue sustains only ~260GB/s -- so
                    # LOAD-BALANCE each bootstrap pair at ~1.78MB per queue:
                    # kt + v-half0 on scalar, qt + v-half1 on sync (the
                    # baseline's kt+v-both on scalar / everything-on-sync
                    # for pair 1 left one queue 4.6us behind the other).
                    # Chunks ship in consumption order (dc-major), pair-0
                    # dc0 split in halves so the first matmul waits on only
                    # ~0.25MB.  gpsimd DMAs are software-DGE (slow) and the
                    # tensor queue must stay matmul-only: two queues is it.
                    keng = nc.scalar if p == 0 else nc.sync
                    # q8 (0.64MB) + k8-dc4 on sync, k8-dc0..3 + v on the k
                    # queue, in dc-consumption order; q8-dc0 ships alone so
                    # the first matmul waits on only 0.13MB
                    nc.sync.dma_start(q8t[par][:, 0:1, :], q8[b, :, 0:1, :])
                    keng.dma_start(k8t[par][:, 0:2, :, :],
                                   k8[b, :, 0:2, :, :])
                    nc.sync.dma_start(q8t[par][:, 1:5, :], q8[b, :, 1:5, :])
                    keng.dma_start(k8t[par][:, 2:4, :, :],
                                   k8[b, :, 2:4, :, :])
                    nc.sync.dma_start(k8t[par][:, 4:5, :, :],
                                      k8[b, :, 4:5, :, :])
                    for half in range(2):
                        keng.dma_start(vt[par][half][:], v[h, b, half])
                else:
                    # steady state: DMA dispatch costs ~620ns of sync-engine
                    # time per instruction regardless of size, so ship each
                    # tensor in as few instructions as possible (prefetch is
                    # a full pair ahead; nothing waits on these directly)
                    nc.sync.dma_start(qt[par][:], qT[h, b])
                    nc.sync.dma_start(kt[par][:], kT[h, b])
                    for half in range(2):
                        nc.sync.dma_start(vt[par][half][:], v[h, b, half])

                # ---- PV for q-tile j (needs pt[kc<=j], emitted after the
                # S chunk for k-chunk j+1 so exp/mask latency is hidden) --
                def emit_pv(j):
                    o_ps = ps_o.tile([128, 1024], f32, tag="o",
                                     name=f"o_ps_{p}_{j}")
                    last_pair = p == HEADS_PER_CORE * BATCH - 1
                    if last_pair and j == KT - 1:
                        # Final tile: run the two PV column-halves as
                        # separate kc chains so half A's recip+mul+DMA all
                        # overlap half B's matmuls, and fan the remaining
                        # normalize/ship across scalar+gpsimd and three DMA
                        # queues -- shortens the exposed tail after the last
                        # matmul from ~3.7us to ~1.5us.
                        row0 = tok0 + j * 128
                        recip = opool.tile([128, 1], f32, tag="recip",
                                           name=f"recip_{p}_{j}")
                        o_sb = opool.tile([128, DV], f16, tag="osb",
                                          name=f"o_sb_{p}_{j}")
                        for kc in range(j + 1):
                            off = 128 * (j - kc)
                            nc.tensor.matmul(
                                o_ps[:, 0:257],
                                lhsT=pt[par][kc][:, off:off + 128],
                                rhs=vt[par][kc // 4][:, kc % 4, 0:257],
                                start=(kc == 0), stop=(kc == j),
                                skip_group_check=True,
                            )
                        nc.vector.reciprocal(recip[:], o_ps[:, 0:1])
                        nc.vector.tensor_scalar_mul(o_sb[:, 0:256],
                                                    o_ps[:, 1:257], recip[:])
                        nc.sync.dma_start(o[h, row0:row0 + 128, 0:256],
                                          o_sb[:, 0:256])
                        # half B accumulates in a borrowed ps_s tile:
                        # Tile tracks dependencies per-tile, so reusing o_ps
                        # would serialize half B's matmuls behind half A's
                        # recip/mul reads.
                        o_ps_b = ps_s.tile([128, 512], f32, tag="s",
                                           name=f"o_ps_b_{p}")
                        for kc in range(j + 1):
                            off = 128 * (j - kc)
                            nc.tensor.matmul(
                                o_ps_b[:, 0:256],
                                lhsT=pt[par][kc][:, off:off + 128],
                                rhs=vt[par][kc // 4][:, kc % 4, 257:513],
                                start=(kc == 0), stop=(kc == j),
                                skip_group_check=True,
                            )
                        # gpsimd cannot read PSUM; vector is free again by
                        # now (its half-A mul ran under half B's matmuls).
                        nc.vector.tensor_scalar_mul(o_sb[:, 256:384],
                                                    o_ps_b[:, 0:128],
                                                    recip[:])
                        nc.sync.dma_start(o[h, row0:row0 + 128, 256:384],
                                          o_sb[:, 256:384])
                        nc.scalar.mul(o_sb[:, 384:512], o_ps_b[:, 128:256],
                                      recip[:])
                        nc.scalar.dma_start(o[h, row0:row0 + 128, 384:512],
                                            o_sb[:, 384:512])
                        return
                    for kc in range(j + 1):
                        off = 128 * (j - kc)
                        lhsT = pt[par][kc][:, off:off + 128]
                        vv = vt[par][kc // 4]
                        nc.tensor.matmul(
                            o_ps[:, 0:257], lhsT=lhsT,
                            rhs=vv[:, kc % 4, 0:257],
                            start=(kc == 0), stop=(kc == j),
                            skip_group_check=True,
                        )
                        nc.tensor.matmul(
                            o_ps[:, 512:768], lhsT=lhsT,
                            rhs=vv[:, kc % 4, 257:513],
                            start=(kc == 0), stop=(kc == j),
                            skip_group_check=True,
                        )
                    recip = opool.tile([128, 1], f32, tag="recip",
                                       name=f"recip_{p}_{j}")
                    nc.vector.reciprocal(recip[:], o_ps[:, 0:1])
                    o_sb = opool.tile([128, DV], f16, tag="osb",
                                      name=f"o_sb_{p}_{j}")
                    # split the normalization across vector and scalar so
                    # neither engine's queue becomes the pair bottleneck
                    nc.vector.tensor_scalar_mul(o_sb[:, 0:256],
                                                o_ps[:, 1:257], recip[:])
                    nc.scalar.mul(o_sb[:, 256:512], o_ps[:, 512:768],
                                  recip[:])
                    row0 = tok0 + j * 128
                    if p == HEADS_PER_CORE * BATCH - 1:
                        # last pair: ship each half as soon as its mul is
                        # done, shortening the final DMA drain
                        nc.sync.dma_start(o[h, row0:row0 + 128, 0:256],
                                          o_sb[:, 0:256])
                        nc.sync.dma_start(o[h, row0:row0 + 128, 256:512],
                                          o_sb[:, 256:512])
                    else:
                        nc.sync.dma_start(o[h, row0:row0 + 128, :], o_sb[:])

                # ---- S^T + exp -> P^T, interleaved with PV -----------
                for g in range(KT):
                    ch = _s_chunks(g)
                    s_tiles = [
                        ps_s.tile([128, 512], f32, tag="s",
                                  name=f"s_{p}_{g}_{qs}")
                        for (qs, w) in ch
                    ]
                    # Pre-start warm-up: ~3us of zero-weight dummies into
                    # the first S tile so the PE ramps to full clock while
                    # the bootstrap DMAs land; the first real matmul then
                    # queues behind them with the clock already at 2.4 GHz.
                    warmed = p == 0 and g == 0 and WARM_PRE > 0
                    if warmed:
                        for i in range(WARM_PRE):
                            nc.tensor.matmul(
                                s_tiles[0][:, 0:512],
                                lhsT=warm[:, 0:128], rhs=warm[:, 0:512],
                                start=(i == 0), stop=False,
                                skip_group_check=True,
                            )
                    # d-chunk outer: one fresh weight load per (g, dc),
                    # streamed over both q column chunks back-to-back.
                    for dc in range(DC):
                        for ci, (qs, w) in enumerate(ch):
                            st = dc == 0 and not (warmed and ci == 0)
                            if p <= 1:
                                qap = q8t[par][:, dc, qs:qs + w]
                                nc.tensor.matmul(
                                    s_tiles[ci][:, :w],
                                    lhsT=k8t[par][:, dc, :,
                                                  128 * g:128 * (g + 1)],
                                    rhs=type(qap)(
                                        qap.tensor, qap.offset,
                                        [list(qap.ap[0]), [0, 2],
                                         list(qap.ap[1])]),
                                    perf_mode=mybir.MatmulPerfMode.DoubleRow,
                                    start=st, stop=(dc == DC - 1),
                                    skip_group_check=True,
                                )
                            else:
                                nc.tensor.matmul(
                                    s_tiles[ci][:, :w],
                                    lhsT=kt[par][:, dc,
                                                 128 * g:128 * (g + 1)],
                                    rhs=qt[par][:, dc, qs:qs + w],
                                    start=st, stop=(dc == DC - 1),
                                    skip_group_check=True,
                                )
                        # Spacer dummies keep the PE continuously busy (and
                        # clocked) through pair-0's DMA-paced S phase: g=0/1
                        # consume q/k faster than HBM delivers them.
                        if p == 0 and g == 0 and dc == 0 and SPACER:
                            for i in range(SPACER):
                                nc.tensor.matmul(
                                    s_tiles[0][:, 0:512],
                                    lhsT=warm[:, 0:128], rhs=warm[:, 0:512],
                                    start=False, stop=False,
                                    skip_group_check=True,
                                )
                    for ci, (qs, w) in enumerate(ch):
                        col0 = qs - 128 * g
                        nc.scalar.activation(
                            pt[par][g][:, col0:col0 + w],
                            s_tiles[ci][:, :w],
                            mybir.ActivationFunctionType.Exp,
                            scale=SCALE,
                        )
                        if ci == 0:
                            # causal mask on the diagonal 128x128 block, in
                            # place on the (otherwise idle) gpsimd engine:
                            # row x = local k, col y = local q; keep iff
                            # x <= y.  Emitted between the two exp chunks:
                            # it only reads chunk 0's columns, and the PV
                            # chain's diagonal-block LDWEIGHTS was measured
                            # waiting ~0.3-0.5us on this mask when it was
                            # queued behind the second exp.
                            nc.gpsimd.affine_select(
                                out=pt[par][g][:, 0:128],
                                in_=pt[par][g][:, 0:128],
                                compare_op=mybir.AluOpType.is_ge,
                                fill=0.0,
                                base=0,
                                pattern=[[1, 128]],
                                channel_multiplier=-1,
                            )
                    if g >= 1:
                        emit_pv(g - 1)
                emit_pv(KT - 1)
    _split_multi_waits(nc)
    return nc


def kernel(q, k, v, cu_seqlens):
    global _CACHED_NC
    from concourse import bass_utils

    # host-side numpy immediately: slicing jax arrays would dispatch XLA
    # ops onto the accelerator platform
    q = np.asarray(q)
    k = np.asarray(k)
    v = np.asarray(v)
    assert q.shape == (TOTAL, NUM_HEADS, HEAD_DIM)
    expected_cu = np.arange(BATCH + 1, dtype=np.int64) * SEQ
    assert np.array_equal(np.asarray(cu_seqlens, dtype=np.int64), expected_cu), (
        f"kernel hardcodes equal {SEQ}-token segments, got {cu_seqlens}"
    )

    if _CACHED_NC is None:
        _CACHED_NC = _build_nc()
    nc = _CACHED_NC

    in_maps = []
    for i in range(N_CORES):
        hs = slice(i * HEADS_PER_CORE, (i + 1) * HEADS_PER_CORE)
        def _qk_layout(x):
            # [T, H, 576] -> [H, B, 128, 5, SEQ], dims zero-padded to 640
            xp = np.zeros((TOTAL, HEADS_PER_CORE, 640), np.float16)
            xp[:, :, :HEAD_DIM] = x
            xp = xp.reshape(BATCH, SEQ, HEADS_PER_CORE, 5, 128)
            return np.ascontiguousarray(xp.transpose(2, 0, 4, 3, 1))

        def _v_layout(x):
            # [T, H, 512] -> [H, B, 2, 128, 4, 513] with ones col 0
            vv = np.concatenate(
                [np.ones((TOTAL, HEADS_PER_CORE, 1), np.float16),
                 x.astype(np.float16)], axis=2)
            vv = vv.reshape(BATCH, 2, 4, 128, HEADS_PER_CORE, DV + 1)
            return np.ascontiguousarray(vv.transpose(4, 0, 1, 3, 2, 5))

        import ml_dtypes
        F8 = ml_dtypes.float8_e4m3

        qTl = _qk_layout(q[:, hs, :])
        kTl = _qk_layout(k[:, hs, :])
        k16 = kTl[0, 0:2].astype(np.float32)          # [2, 128, 5, SEQ]
        khi = k16.astype(F8)
        klo = (k16 - khi.astype(np.float32)).astype(F8)
        in_maps.append({
            "qT": qTl,
            "kT": kTl,
            "q8": qTl[0, 0:2].astype(F8),
            "k8": np.ascontiguousarray(
                np.stack([khi, klo], axis=3)),         # [2,128,5,2,SEQ]
            "v": _v_layout(v[:, hs, :DV]),
        })

    res = bass_utils.run_bass_kernel_spmd(nc, in_maps,
                                          core_ids=list(range(N_CORES)))
    globals()["_LAST_RESULTS"] = res
    globals()["_LAST_EXEC_NS"] = res.exec_time_ns

    out = np.empty((TOTAL, NUM_HEADS, DV), dtype=np.float32)
    for i in range(N_CORES):
        hs = slice(i * HEADS_PER_CORE, (i + 1) * HEADS_PER_CORE)
        out[:, hs, :] = res.results[i]["o"].transpose(1, 0, 2).astype(
            np.float32)
    return out



# revision 41
# speedup vs baseline: 1.0099x; 1.0099x over previous
"""Varlen causal attention (MLA-style) for trn2, sharded over 8 NeuronCores.

Problem: q,k,v [4096, 16, 576] fp32, 4 equal packed sequences of 1024 tokens,
causal attention per sequence per head, output sliced to [..., :512].

Sharding: tensor-parallel over heads — 2 heads per core, all 4 sequences.
Per (head, seq) pair the kernel computes S^T = K @ Q^T directly in
[k-partition, q-free] orientation so that P^T = exp(S^T * scale) is already
the stationary operand layout needed by the PV matmul (O = P^T.T @ V), and V
is used in its natural [token, dv] layout.  Softmax max-subtraction is skipped
(scores are ~N(0,1), |s| < ~6, exp is well-conditioned in fp32); the
denominator falls out of the PV matmul itself: v ships with a leading ones
column and PV is split 257+256 so neither matmul crosses a PSUM bank --
output column 0 is the softmax denominator, at zero extra matmuls.

Schedule (from trace analysis across iterations):
 * S^T and PV are interleaved within a pair (PV for q-tile g-1 is emitted
   right after the S^T chunk for k-chunk g), which removes the ~8us
   exposed PV+softmax tail after the last pair's S phase.
 * S^T runs d-chunk-outer so each kt weight tile streams both q column
   chunks back-to-back (weight reloads between back-to-back matmuls are
   ~5ns vs ~50ns for fresh loads at accumulation-group boundaries).
 * Input dram layouts EXACTLY mirror the SBUF tiles ([128 partitions, 5
   d-chunks, 1024 toks] per (head, seq) pair, rope dims zero-padded on the
   host), so each steady-state tensor ships as ONE DMA whose descriptors
   are full 10KB per-partition runs, and no on-device memsets are needed.
 * The startup is DMA-bound (~2.5MB of pair-0 q/k must land before S-g0
   can finish, vs ~150GB/s/queue early): the PE clock also ramps
   0.65->1.2->2.4GHz over ~3.5us of continuous execution and idle gaps
   drop it back.  So WARM_PRE zero-weight dummy matmuls start the PE at
   ~7.6us (right after the NEFF preamble, accumulating +0 into the live
   first S tile) paying the ramp on throwaway work during the otherwise
   dead DMA window, and SPACER bridge dummies after the dc0 pass keep the
   clock alive across the dc1-4 starvation stretch.  Short (~1us) gaps
   after that cost only a ~0.2us p-state hiccup.
 * DMA dispatch costs ~620ns of issuing-engine occupancy per instruction,
   so steady-state inputs ship as 4 large DMAs per pair on the otherwise
   idle sync queue (never scalar: the exps live there and PV stalls
   behind dispatches; never gpsimd: software-DGE, much slower).  Pair
   0/1 bootstrap splits kt+v across scalar and qt across sync in
   dc-consumption order.
 * o_sb/recip get 8 buffers: output-DMA completion lags dispatch by ~3us,
   and with shallow buffering the vector muls block on that WAR, which
   backs up into PSUM recycling and stalls PV starts.
 * The final q-tile's PV runs as two separate column-half chains into
   separate PSUM tiles, so half A's recip+mul+DMA overlap half B's
   matmuls and the exposed tail shrinks by ~1us; the remaining muls fan
   over vector+scalar with same-engine DMA dispatch.
 * The output is stored as fp16 (the reference is fp32 but the harness
   tolerance is 2e-2; fp16 rounding adds ~1e-4) and upcast on the host,
   halving output HBM traffic.

Host-side prep per core: v is shipped as [ones | v 0:512] per 512-token
half so the PV matmul produces the softmax denominator as output column 0
for free; all inputs are cast to fp16 (the PE runs fp16 matmuls at 1
cycle/col vs 4 for fp32; end-to-end relative error ~4e-4, PSUM fp32).

Measured: ~163.5us on 8 trn2 cores at full clock (fp16-only variant:
164.3-166.0us; session baseline: 165.9-166.8us), rel err 1.147e-02
(deterministic: fixed inputs, one-sided fp8 on 1/4 of output rows).
PE-streaming roofline for fp16 is ~138.4us/core, ~9us fixed NEFF
preamble+DMA flow-start, ~3.4us per-instruction overhead, ~5us
tail+postamble.  NOTE the device itself sometimes runs throttled at
2.0GHz instead of 2.4 (every matmul uniformly 1.2x slower, whole-run
~+30us) -- compare configurations only at equal clock (512-col matmul
min duration: 215ns = full clock, 258ns = throttled).  fp8 DoubleRow
compute streams at 1 cycle/col on HW (2x contraction depth, not 2x
rate), so fp8 cannot beat fp16 on PE time within the 2e-2 error budget
-- its value here is purely the halved startup DMA bytes for the
DMA-bound bootstrap pairs.
"""

import sys

if "/opt/trn_rl_repo" not in sys.path:
    sys.path.insert(0, "/opt/trn_rl_repo")

import numpy as np

NUM_HEADS = 16
HEAD_DIM = 576
DV = 512
BATCH = 4
SEQ = 1024
TOTAL = BATCH * SEQ
N_CORES = 8
HEADS_PER_CORE = NUM_HEADS // N_CORES  # 2
SCALE = float(1.0 / np.float32(np.sqrt(np.float32(HEAD_DIM))))

_CACHED_NC = None
KT0 = SEQ // 128  # k-chunks per sequence

# PE warm-up dummy matmul counts (see _build_nc): pre-start dummies cover
# the DMA bootstrap window (~6.4us queue-start to ~10.5us data-ready, with
# the first few at the ramping 0.65/1.2 GHz clock); spacers bridge the
# DMA-paced stretches inside pair-0's S phase.
WARM_PRE = 9
SPACER = 10


def _split_multi_waits(nc):
    """The trn2 TPB ISA carries a single sync-wait slot per instruction;
    Tile's sem assignment can emit several.  Hoist excess waits onto
    freshly-inserted NOPs on the same engine immediately before the
    instruction (identical semantics: the engine queue stalls on the NOPs
    first, then the instruction itself)."""
    import concourse.mybir as mybir

    nop_id = 0
    for fn in nc.m.functions:
        for bb in fn.blocks:
            insts = bb.instructions
            i = 0
            while i < len(insts):
                inst = insts[i]
                si = inst.sync_info
                if si is not None and si.on_wait and len(si.on_wait) > 1:
                    waits = list(si.on_wait)
                    si.on_wait = waits[:1]
                    nops = []
                    for w in waits[1:]:
                        nop = mybir.InstNoOp(
                            name=f"bass_waitsplit_{nop_id}",
                            engine=inst.engine,
                            bass_nofuse=True,
                            sync_info=mybir.SyncInfo(on_wait=[w], on_update=[]),
                        )
                        nop_id += 1
                        nc.register_instruction(nop, overwrite=True)
                        nops.append(nop)
                    insts[i:i] = nops
                    i += len(nops)
                i += 1


def _s_chunks(g):
    """Column chunks (qs, w) covering q cols [128*g, SEQ) in <=512-wide
    pieces, rebalancing a would-be 128 remainder into the previous chunk
    (640 -> 384+256, not 512+128: sub-256-col matmuls are LDWEIGHTS-bound)."""
    out = []
    qs = 128 * g
    while qs < SEQ:
        rem = SEQ - qs
        if rem > 512 and rem - 512 < 256:
            w = rem - 256
        else:
            w = min(512, rem)
        out.append((qs, w))
        qs += w
    return out


def _build_nc():
    """Build the per-core Bass module (same NEFF on all 8 cores)."""
    import concourse.bass as bass
    import concourse.mybir as mybir
    import concourse.tile as tile

    f32 = mybir.dt.float32
    f16 = mybir.dt.float16
    f8 = mybir.dt.float8e4
    nc = bass.Bass("TRN2", target_bir_lowering=False, debug=False)

    # Input dram layouts EXACTLY match the destination SBUF tiles: one
    # (head, seq) pair-tensor is [128 partitions, 5 dc-chunks, 1024 toks]
    # with each partition's 10KB fully contiguous (rope dims 512:576 live in
    # chunk 4 partitions 0:64, partitions 64:128 are host zeros).  DMA rings
    # process per-contiguous-run descriptors, so matching layouts turn five
    # 2KB runs per partition into one 10KB run -- the descriptor rate, not
    # HBM bandwidth, paces the DMA-bound startup.
    qT = nc.dram_tensor("qT", [HEADS_PER_CORE, BATCH, 128, 5, SEQ], f16,
                        kind="ExternalInput").ap()
    kT = nc.dram_tensor("kT", [HEADS_PER_CORE, BATCH, 128, 5, SEQ], f16,
                        kind="ExternalInput").ap()
    # Bootstrap pairs 0/1 (head 0, seqs 0/1) ship q as SINGLE fp8 e4m3
    # (half the startup-critical bytes; the DMA-bound first ~18us is the
    # kernel's largest overhead) and k as exact (hi, lo) fp8 planes.  Their
    # S matmuls run as one-sided DoubleRow: lhsT = k (hi,lo) pairs, rhs = a
    # stride-0 AP that streams each q8 column twice -- so the only
    # quantization error is q8's one-sided 2.65% on 2/8 pairs per core
    # = ~1.15e-2 end-to-end vs the 2e-2 gate.  DoubleRow streams at 1
    # cycle/col on trn2 (2x depth, not rate): same PE time as fp16.
    q8 = nc.dram_tensor("q8", [2, 128, 5, SEQ], f8,
                        kind="ExternalInput").ap()
    k8 = nc.dram_tensor("k8", [2, 128, 5, 2, SEQ], f8,
                        kind="ExternalInput").ap()
    # v ships with a leading ones column: the PV matmul then produces the
    # softmax denominator as output column 0 for free (split 257+256 so
    # neither matmul crosses a PSUM bank).  Layout matches vt tiles:
    # [half, 128 partitions, 4 k-chunks, 513] -- 4.1KB runs.
    v = nc.dram_tensor("v", [HEADS_PER_CORE, BATCH, 2, 128, KT0 // 2,
                             DV + 1], f16, kind="ExternalInput").ap()
    o = nc.dram_tensor("o", [HEADS_PER_CORE, TOTAL, DV], f16,
                       kind="ExternalOutput").ap()

    KT = SEQ // 128     # 8 k-chunks of 128 per sequence
    DC = 5              # d chunks: 4 x 128 + 1 x 64-padded-to-128

    with tile.TileContext(nc) as tc:
        with (
            tc.tile_pool(name="const", bufs=1) as cpool,
            tc.tile_pool(name="outp", bufs=8) as opool,
            tc.tile_pool(name="ps_s", bufs=4, space="PSUM") as ps_s,
            tc.tile_pool(name="ps_o", bufs=2, space="PSUM") as ps_o,
        ):
            # Persistent ping-pong input tiles (explicit parity instead of
            # pool rotation) so the rope-pad rows can be zeroed exactly once.
            qt = [cpool.tile([128, DC, SEQ], f16, tag=f"qt_{par}",
                             name=f"qt_{par}")
                  for par in range(2)]
            kt = [cpool.tile([128, DC, SEQ], f16, tag=f"kt_{par}",
                             name=f"kt_{par}")
                  for par in range(2)]
            q8t = [cpool.tile([128, 5, SEQ], f8, tag=f"q8_{par}",
                              name=f"q8_{par}")
                   for par in range(2)]
            k8t = [cpool.tile([128, 5, 2, SEQ], f8, tag=f"k8_{par}",
                              name=f"k8_{par}")
                   for par in range(2)]
            vt = [[cpool.tile([128, KT // 2, DV + 1], f16,
                              tag=f"v{half}_{par}", name=f"v{half}_{par}")
                   for half in range(2)]
                  for par in range(2)]
            # P^T per k-chunk, causal width, persists across the whole pair.
            pt = [[cpool.tile([128, SEQ - 128 * g], f16, tag=f"pt{g}_{par}",
                              name=f"pt{g}_{par}")
                   for g in range(KT)] for par in range(2)]

            # Warm-up source for zero-weight dummy matmuls (see below): the
            # PE clock p-states ramp 0.65 -> 1.2 -> 2.4 GHz over ~3us of
            # CONTINUOUS execution, and any idle gap resets the ramp, so the
            # first ~10us of real matmuls otherwise run at half clock while
            # the bootstrap DMAs land (~4us lost) and each starvation gap
            # costs its length again in re-ramp.  Dummy matmuls with zero
            # weights accumulate +0 into the live S PSUM tile: they burn PE
            # time without changing results, paying the ramp on throwaway
            # work and bridging pair-0's DMA-paced stretches.  memset on
            # gpsimd: its queue issues first (~6.3us), vector's ~1.5us later.
            warm = None
            if WARM_PRE or SPACER:
                warm = cpool.tile([128, 512], f16, tag="warm", name="warm")
                nc.gpsimd.memset(warm[:], 0.0)

            for p in range(HEADS_PER_CORE * BATCH):
                h, b = divmod(p, BATCH)
                par = p % 2
                tok0 = b * SEQ

                # ---- input DMAs --------------------------------------
                # All inputs on the (otherwise idle) sync queue, per-128-row
                # chunks interleaved in consumption order so the first S
                # matmuls wait on ~0.5MB instead of the whole 2.4MB pair.
                # DMA dispatch costs ~600ns of issuing-engine occupancy, so
                # input DMAs must NOT share a queue with the exps (scalar).
                # pair 0 bootstraps on both HWDGE queues (scalar is still
                # empty of exps at that point) so the two descriptor
                # generators fill SBUF in parallel
                if p <= 1:
                    # bootstrap: the startup is paced by DMA supply, and
                    # a single HWDGE queue sustains only ~260GB/s -- so
                    # LOAD-BALANCE each bootstrap pair at ~1.78MB per queue:
                    # kt + v-half0 on scalar, qt + v-half1 on sync (the
                    # baseline's kt+v-both on scalar / everything-on-sync
                    # for pair 1 left one queue 4.6us behind the other).
                    # Chunks ship in consumption order (dc-major), pair-0
                    # dc0 split in halves so the first matmul waits on only
                    # ~0.25MB.  gpsimd DMAs are software-DGE (slow) and the
                    # tensor queue must stay matmul-only: two queues is it.
                    keng = nc.scalar if p == 0 else nc.sync
                    # q8 (0.64MB) + k8-dc4 on sync, k8-dc0..3 + v on the k
                    # queue, in dc-consumption order; q8-dc0 ships alone so
                    # the first matmul waits on only 0.13MB
                    nc.sync.dma_start(q8t[par][:, 0:1, :], q8[b, :, 0:1, :])
                    keng.dma_start(k8t[par][:, 0:2, :, :],
                                   k8[b, :, 0:2, :, :])
                    nc.sync.dma_start(q8t[par][:, 1:5, :], q8[b, :, 1:5, :])
                    keng.dma_start(k8t[par][:, 2:4, :, :],
                                   k8[b, :, 2:4, :, :])
                    nc.sync.dma_start(k8t[par][:, 4:5, :, :],
                                      k8[b, :, 4:5, :, :])
                    for half in range(2):
                        keng.dma_start(vt[par][half][:], v[h, b, half])
                else:
                    # steady state: DMA dispatch costs ~620ns of sync-engine
                    # time per instruction regardless of size, so ship each
                    # tensor in as few instructions as possible (prefetch is
                    # a full pair ahead; nothing waits on these directly)
                    nc.sync.dma_start(qt[par][:], qT[h, b])
                    nc.sync.dma_start(kt[par][:], kT[h, b])
                    for half in range(2):
                        nc.sync.dma_start(vt[par][half][:], v[h, b, half])

                # ---- PV for q-tile j (needs pt[kc<=j], emitted after the
                # S chunk for k-chunk j+1 so exp/mask latency is hidden) --
                def emit_pv(j):
                    o_ps = ps_o.tile([128, 1024], f32, tag="o",
                                     name=f"o_ps_{p}_{j}")
                    last_pair = p == HEADS_PER_CORE * BATCH - 1
                    if last_pair and j == KT - 1:
                        # Final tile: run the two PV column-halves as
                        # separate kc chains so half A's recip+mul+DMA all
                        # overlap half B's matmuls, and fan the remaining
                        # normalize/ship across scalar+gpsimd and three DMA
                        # queues -- shortens the exposed tail after the last
                        # matmul from ~3.7us to ~1.5us.
                        row0 = tok0 + j * 128
                        recip = opool.tile([128, 1], f32, tag="recip",
                                           name=f"recip_{p}_{j}")
                        o_sb = opool.tile([128, DV], f16, tag="osb",
                                          name=f"o_sb_{p}_{j}")
                        for kc in range(j + 1):
                            off = 128 * (j - kc)
                            nc.tensor.matmul(
                                o_ps[:, 0:257],
                                lhsT=pt[par][kc][:, off:off + 128],
                                rhs=vt[par][kc // 4][:, kc % 4, 0:257],
                                start=(kc == 0), stop=(kc == j),
                                skip_group_check=True,
                            )
                        nc.vector.reciprocal(recip[:], o_ps[:, 0:1])
                        nc.vector.tensor_scalar_mul(o_sb[:, 0:256],
                                                    o_ps[:, 1:257], recip[:])
                        nc.sync.dma_start(o[h, row0:row0 + 128, 0:256],
                                          o_sb[:, 0:256])
                        # half B accumulates in a borrowed ps_s tile:
                        # Tile tracks dependencies per-tile, so reusing o_ps
                        # would serialize half B's matmuls behind half A's
                        # recip/mul reads.
                        o_ps_b = ps_s.tile([128, 512], f32, tag="s",
                                           name=f"o_ps_b_{p}")
                        for kc in range(j + 1):
                            off = 128 * (j - kc)
                            nc.tensor.matmul(
                                o_ps_b[:, 0:256],
                                lhsT=pt[par][kc][:, off:off + 128],
                                rhs=vt[par][kc // 4][:, kc % 4, 257:513],
                                start=(kc == 0), stop=(kc == j),
                                skip_group_check=True,
                            )
                        # gpsimd cannot read PSUM; vector is free again by
                        # now (its half-A mul ran under half B's matmuls).
                        nc.vector.tensor_scalar_mul(o_sb[:, 256:384],
                                                    o_ps_b[:, 0:128],
                                                    recip[:])
                        nc.sync.dma_start(o[h, row0:row0 + 128, 256:384],
                                          o_sb[:, 256:384])
                        nc.scalar.mul(o_sb[:, 384:512], o_ps_b[:, 128:256],
                                      recip[:])
                        nc.scalar.dma_start(o[h, row0:row0 + 128, 384:512],
                                            o_sb[:, 384:512])
                        return
                    for kc in range(j + 1):
                        off = 128 * (j - kc)
                        lhsT = pt[par][kc][:, off:off + 128]
                        vv = vt[par][kc // 4]
                        nc.tensor.matmul(
                            o_ps[:, 0:257], lhsT=lhsT,
                            rhs=vv[:, kc % 4, 0:257],
                            start=(kc == 0), stop=(kc == j),
                            skip_group_check=True,
                        )
                        nc.tensor.matmul(
                            o_ps[:, 512:768], lhsT=lhsT,
                            rhs=vv[:, kc % 4, 257:513],
                            start=(kc == 0), stop=(kc == j),
                            skip_group_check=True,
                        )
                    recip = opool.tile([128, 1], f32, tag="recip",
                                       name=f"recip_{p}_{j}")
                    nc.vector.reciprocal(recip[:], o_ps[:, 0:1])
                    o_sb = opool.tile([128, DV], f16, tag="osb",
                                      name=f"o_sb_{p}_{j}")
                    # split the normalization across vector and scalar so
                    # neither engine's queue becomes the pair bottleneck
                    nc.vector.tensor_scalar_mul(o_sb[:, 0:256],
                                                o_ps[:, 1:257], recip[:])
                    nc.scalar.mul(o_sb[:, 256:512], o_ps[:, 512:768],
                                  recip[:])
                    row0 = tok0 + j * 128
                    if p == HEADS_PER_CORE * BATCH - 1:
                        # last pair: ship each half as soon as its mul is
                        # done, shortening the final DMA drain
                        nc.sync.dma_start(o[h, row0:row0 + 128, 0:256],
                                          o_sb[:, 0:256])
                        nc.sync.dma_start(o[h, row0:row0 + 128, 256:512],
                                          o_sb[:, 256:512])
                    else:
                        nc.sync.dma_start(o[h, row0:row0 + 128, :], o_sb[:])

                # ---- S^T + exp -> P^T, interleaved with PV -----------
                for g in range(KT):
                    ch = _s_chunks(g)
                    s_tiles = [
                        ps_s.tile([128, 512], f32, tag="s",
                                  name=f"s_{p}_{g}_{qs}")
                        for (qs, w) in ch
                    ]
                    # Pre-start warm-up: ~3us of zero-weight dummies into
                    # the first S tile so the PE ramps to full clock while
                    # the bootstrap DMAs land; the first real matmul then
                    # queues behind them with the clock already at 2.4 GHz.
                    warmed = p == 0 and g == 0 and WARM_PRE > 0
                    if warmed:
                        for i in range(WARM_PRE):
                            nc.tensor.matmul(
                                s_tiles[0][:, 0:512],
                                lhsT=warm[:, 0:128], rhs=warm[:, 0:512],
                                start=(i == 0), stop=False,
                                skip_group_check=True,
                            )
                    # d-chunk outer: one fresh weight load per (g, dc),
                    # streamed over both q column chunks back-to-back.
                    for dc in range(DC):
                        for ci, (qs, w) in enumerate(ch):
                            st = dc == 0 and not (warmed and ci == 0)
                            if p <= 1:
                                qap = q8t[par][:, dc, qs:qs + w]
                                nc.tensor.matmul(
                                    s_tiles[ci][:, :w],
                                    lhsT=k8t[par][:, dc, :,
                                                  128 * g:128 * (g + 1)],
                                    rhs=type(qap)(
                                        qap.tensor, qap.offset,
                                        [list(qap.ap[0]), [0, 2],
                                         list(qap.ap[1])]),
                                    perf_mode=mybir.MatmulPerfMode.DoubleRow,
                                    start=st, stop=(dc == DC - 1),
                                    skip_group_check=True,
                                )
                            else:
                                nc.tensor.matmul(
                                    s_tiles[ci][:, :w],
                                    lhsT=kt[par][:, dc,
                                                 128 * g:128 * (g + 1)],
                                    rhs=qt[par][:, dc, qs:qs + w],
                                    start=st, stop=(dc == DC - 1),
                                    skip_group_check=True,
                                )
                        # Spacer dummies keep the PE continuously busy (and
                        # clocked) through pair-0's DMA-paced S phase: g=0/1
                        # consume q/k faster than HBM delivers them.
                        if p == 0 and g == 0 and dc == 0 and SPACER:
                            for i in range(SPACER):
                                nc.tensor.matmul(
                                    s_tiles[0][:, 0:512],
                                    lhsT=warm[:, 0:128], rhs=warm[:, 0:512],
                                    start=False, stop=False,
                                    skip_group_check=True,
                                )
                    for ci, (qs, w) in enumerate(ch):
                        col0 = qs - 128 * g
                        nc.scalar.activation(
                            pt[par][g][:, col0:col0 + w],
                            s_tiles[ci][:, :w],
                            mybir.ActivationFunctionType.Exp,
                            scale=SCALE,
                        )
                        if ci == 0:
                            # causal mask on the diagonal 128x128 block, in
                            # place on the (otherwise idle) gpsimd engine:
                            # row x = local k, col y = local q; keep iff
                            # x <= y.  Emitted between the two exp chunks:
                            # it only reads chunk 0's columns, and the PV
                            # chain's diagonal-block LDWEIGHTS was measured
                            # waiting ~0.3-0.5us on this mask when it was
                            # queued behind the second exp.
                            nc.gpsimd.affine_select(
                                out=pt[par][g][:, 0:128],
                                in_=pt[par][g][:, 0:128],
                                compare_op=mybir.AluOpType.is_ge,
                                fill=0.0,
                                base=0,
                                pattern=[[1, 128]],
                                channel_multiplier=-1,
                            )
                    if g >= 1:
                        emit_pv(g - 1)
                emit_pv(KT - 1)
    _split_multi_waits(nc)
    return nc


def kernel(q, k, v, cu_seqlens):
    global _CACHED_NC
    from concourse import bass_utils

    # host-side numpy immediately: slicing jax arrays would dispatch XLA
    # ops onto the accelerator platform
    q = np.asarray(q)
    k = np.asarray(k)
    v = np.asarray(v)
    assert q.shape == (TOTAL, NUM_HEADS, HEAD_DIM)
    expected_cu = np.arange(BATCH + 1, dtype=np.int64) * SEQ
    assert np.array_equal(np.asarray(cu_seqlens, dtype=np.int64), expected_cu), (
        f"kernel hardcodes equal {SEQ}-token segments, got {cu_seqlens}"
    )

    if _CACHED_NC is None:
        _CACHED_NC = _build_nc()
    nc = _CACHED_NC

    in_maps = []
    for i in range(N_CORES):
        hs = slice(i * HEADS_PER_CORE, (i + 1) * HEADS_PER_CORE)
        def _qk_layout(x):
            # [T, H, 576] -> [H, B, 128, 5, SEQ], dims zero-padded to 640
            xp = np.zeros((TOTAL, HEADS_PER_CORE, 640), np.float16)
            xp[:, :, :HEAD_DIM] = x
            xp = xp.reshape(BATCH, SEQ, HEADS_PER_CORE, 5, 128)
            return np.ascontiguousarray(xp.transpose(2, 0, 4, 3, 1))

        def _v_layout(x):
            # [T, H, 512] -> [H, B, 2, 128, 4, 513] with ones col 0
            vv = np.concatenate(
                [np.ones((TOTAL, HEADS_PER_CORE, 1), np.float16),
                 x.astype(np.float16)], axis=2)
            vv = vv.reshape(BATCH, 2, 4, 128, HEADS_PER_CORE, DV + 1)
            return np.ascontiguousarray(vv.transpose(4, 0, 1, 3, 2, 5))

        import ml_dtypes
        F8 = ml_dtypes.float8_e4m3

        qTl = _qk_layout(q[:, hs, :])
        kTl = _qk_layout(k[:, hs, :])
        k16 = kTl[0, 0:2].astype(np.float32)          # [2, 128, 5, SEQ]
        khi = k16.astype(F8)
        klo = (k16 - khi.astype(np.float32)).astype(F8)
        in_maps.append({
            "qT": qTl,
            "kT": kTl,
            "q8": qTl[0, 0:2].astype(F8),
            "k8": np.ascontiguousarray(
                np.stack([khi, klo], axis=3)),         # [2,128,5,2,SEQ]
            "v": _v_layout(v[:, hs, :DV]),
        })

    res = bass_utils.run_bass_kernel_spmd(nc, in_maps,
                                          core_ids=list(range(N_CORES)))
    globals()["_LAST_RESULTS"] = res
    globals()["_LAST_EXEC_NS"] = res.exec_time_ns

    out = np.empty((TOTAL, NUM_HEADS, DV), dtype=np.float32)
    for i in range(N_CORES):
        hs = slice(i * HEADS_PER_CORE, (i + 1) * HEADS_PER_CORE)
        out[:, hs, :] = res.results[i]["o"].transpose(1, 0, 2).astype(
            np.float32)
    return out

